# revision 61
# baseline (speedup 1.0000x reference)
"""Trainium2 Bass kernel for nn_FALayer (gated edge-weighted scatter-sum GNN).

Math (reference):
    w_dst, w_src = gate_w[0,:D], gate_w[0,D:]
    g_e  = tanh(h[dst_e]@w_dst + h[src_e]@w_src + gate_b)
    e_e  = g_e * d[dst_e] * d[src_e]
    z    = segment_sum(h[src_e] * e_e, dst_e, N)          # [N, D]

Device strategy (8 cores, SPMD, edges sorted by dst, node-aligned ranges):
  * HOST-uploaded gather table htab [NT, 256] bf16 (512B rows cost the
    same DMA time as 256B under the <512B descriptor penalty): row r =
    [h[node(r)] | projs-hole | d[node(r)] | pad].  The device only
    computes projs = h@w_src for its shard (PE matmuls on a
    host-transposed slice), AllGathers it (bf16), and fills column 128
    with two strided DMA writes (grid-A rows first).  No 25MB
    table-build round trip.
  * Per 512-dst window: int16 dma_gathers chunked to <=1024 idxs/call
    (device limit); grid A = table rows [0,32768), grid B =
    [NT-32768, NT) via an offset view.  Mid rows are assignable to
    either grid, so the per-chunk class split stays balanced and chunk
    overflow routes to C-B columns only (CA=0 saves a gather call).
  * Fixed 32-node chunks, B_A=B_B=2 columns per chunk (cap 256+256 vs
    mean 512 edges); overflow spills to per-window C-B columns matched
    with a 512-wide one-hot.  Column counts are per-window (CB_W mostly
    2; the final partial window only covers ceil(range%512 / 32)
    chunks), trimming gather padding.
  * One-hots built TRANSPOSED [P, c, col] with col = o*cpw + chunk and
    materialized full iotas, so every DVE tensor_tensor operand keeps a
    packed 2-byte last dim (2x mode).  projd row is produced in both
    chunk-transposed [P, c, g] and C-replicated [P, pos, k] layouts on
    the Activation engine.
  * Per-slot gate: projd selected by one-hot mult+reduce, projs and
    d_src read from the gathered row, d_dst as a per-column scale on
    z^T (output stored bf16, cast on host).
  * Scatter-sum via TensorE: z^T[:, chunk] += GH[slots,:].T @ (OneHot*g).
"""

import sys

import numpy as np

sys.path.insert(0, "/opt/trn_rl_repo")

import ml_dtypes  # noqa: E402

import concourse.bass as bass  # noqa: E402
import concourse.bass_isa as bass_isa  # noqa: E402
import concourse.tile as tile  # noqa: E402
from concourse import mybir  # noqa: E402
from concourse.bass_utils import run_bass_kernel_spmd  # noqa: E402

NCORES = 8
P = 128
D = 128
ROWE = 256        # table row width in bf16 elems (512B)
CH = 32           # psum columns per chunk
CPW = 16          # chunks per window
WCOLS = CH * CPW  # 512
B_A = 2           # grid-A columns per chunk (cap 256 edges)
B_B = 2           # grid-B columns per chunk (cap 256 edges)
SPLIT = 32768     # int16 index range (grid A: rows [0,32768))
DUMMY = 999.0
F32 = mybir.dt.float32
F16 = mybir.dt.float16
BF16 = mybir.dt.bfloat16
I16 = mybir.dt.int16
AF = mybir.ActivationFunctionType
OP = mybir.AluOpType


def _split_multiwait(nc):
    """This container's walrus build only accepts one sync-wait per
    instruction; hoist extras onto standalone waits on the same engine."""
    dummy = nc.alloc_semaphore("waitsplit_dummy")
    for f in nc.m.functions:
        for bb in f.blocks:
            il = bb.instructions
            i = 0
            while i < len(il):
                inst = il[i]
                si = inst.sync_info
                if si is not None and len(si.on_wait) > 1:
                    waits = list(si.on_wait)
                    si.on_wait = [waits[-1]]
                    for w in waits[:-1]:
                        binst = nc.engines[inst.engine].wait_ge(dummy, 0)
                        raw = binst.ins
                        owner = nc.cur_bb.bb.instructions
                        assert owner[-1] is raw
                        owner.pop()
                        raw.sync_info.on_wait = [w]
                        il.insert(i, raw)
                        i += 1
                i += 1


def _fix_library_reloads(nc):
    """Insert GPSIMD library loads (Bacc pass) and give the pseudo reload
    markers real PSEUDO_INST bytes so walrus codegen accepts them."""
    from concourse.bacc import Bacc
    Bacc.insert_library_loads(nc)
    isa = nc.isa
    e = isa.get_enum("NEURON_ISA_TPB_PSEUDO_OPCODE")
    pop = e.NEURON_ISA_TPB_PSEUDO_OPCODE_PSEUDO_LIBRARY_RELOAD_INDEX.value
    for f in nc.m.functions:
        for bb in f.blocks:
            for inst in bb.instructions:
                if type(inst).__name__ == "InstPseudoReloadLibraryIndex":
                    if not inst.instr:
                        instr, _ = bass_isa.isa_struct(
                            isa, isa.Opcode.NEURON_ISA_TPB_OPCODE_PSEUDO_INST,
                            {"pseudo_opcode": pop, "lib_index": inst.lib_index})
                        inst.instr = instr


def _wrap16(idx, n):
    """int16 index list -> [128, n/16]: value i at [i%16, i//16],
    replicated to all 8 Q7-core partition groups."""
    blk = np.zeros((16, n // 16), np.int16)
    ar = np.arange(len(idx))
    blk[ar % 16, ar // 16] = idx
    return np.tile(blk, (8, 1))


def _preprocess(src, dst, N):
    E = src.shape[0]
    order = np.argsort(dst, kind="stable")
    dst_s = dst[order].astype(np.int64)
    src_s = src[order].astype(np.int64)

    SHARD = -(-N // NCORES)            # 6250
    KT = -(-SHARD // P)                # 49
    SHARDP = KT * P                    # 6272
    NT = NCORES * SHARDP               # 50176

    def remap(n):
        return SHARDP * (n // SHARD) + (n % SHARD)

    rsrc = remap(src_s)

    # node-aligned, edge-balanced core cuts
    cut_nodes = [0]
    cut_pos = [0]
    for c in range(1, NCORES):
        pos = c * E // NCORES
        node = int(dst_s[min(pos, E - 1)])
        cut_nodes.append(node)
        cut_pos.append(int(np.searchsorted(dst_s, node, "left")))
    cut_nodes.append(N)
    cut_pos.append(E)
    cut_nodes = np.array(cut_nodes, dtype=np.int64)

    max_range = int((cut_nodes[1:] - cut_nodes[:-1]).max())
    NW = -(-max_range // WCOLS)
    NCHUNK = NW * CPW
    # live chunks per window (the last window covers a partial dst range)
    CPW_W = [min(CPW, max(0, -(-(max_range - WCOLS * w) // CH)))
             for w in range(NW)]

    # grid A gathers table rows [0, SPLIT); grid B rows [NT-SPLIT, NT).
    # mid rows [NT-SPLIT, SPLIT) are reachable from BOTH grids; split each
    # chunk's edges so the two classes stay balanced under the 2+2 caps.
    BBASE = NT - SPLIT
    capA, capB = B_A * P, B_B * P
    per_core_raw = []
    max_ca, max_cb = 0, 1
    for c in range(NCORES):
        n0, n1 = int(cut_nodes[c]), int(cut_nodes[c + 1])
        lo, hi = int(cut_pos[c]), int(cut_pos[c + 1])
        cdst = dst_s[lo:hi]
        crs = rsrc[lo:hi]
        bounds = n0 + CH * np.arange(NCHUNK + 1)
        e_bnd = np.searchsorted(cdst, np.minimum(bounds, max(n1, n0)))
        wins = []
        for w in range(NW):
            wchunks = []
            ovA, ovB = [], []
            for m in range(CPW_W[w]):
                gm = w * CPW + m
                e0, e1 = int(e_bnd[gm]), int(e_bnd[gm + 1])
                rs = crs[e0:e1]
                dl = (cdst[e0:e1] - bounds[gm]).astype(np.int64)
                low = rs < BBASE
                high = rs >= SPLIT
                mid = ~(low | high)
                ra = list(zip(rs[low], dl[low]))
                rb = list(zip(rs[high] - BBASE, dl[high]))
                mids = list(zip(rs[mid], dl[mid]))
                # balance classes; when the chunk overflows both caps, split
                # the excess to keep the window's C-A/C-B loads even.
                e = len(ra) + len(rb) + len(mids)
                # balance classes; chunk overflow routes to C-B only
                # (CA=0 saves a gather call and the C-A one-hot ops)
                want = min((e + 1) // 2, capA)
                want_a = min(max(want, len(ra)), len(ra) + len(mids))
                take = want_a - len(ra)
                ra += mids[:take]
                rb += [(r - BBASE, d_) for r, d_ in mids[take:]]
                if len(ra) > capA:   # overflow -> C-A
                    ovA += [(r, d_ + m * CH) for r, d_ in ra[capA:]]
                    ra = ra[:capA]
                if len(rb) > capB:   # overflow -> C-B
                    ovB += [(r, d_ + m * CH) for r, d_ in rb[capB:]]
                    rb = rb[:capB]
                wchunks.append((
                    np.array([r for r, _ in ra], np.int64),
                    np.array([d_ for _, d_ in ra], np.int64),
                    np.array([r for r, _ in rb], np.int64),
                    np.array([d_ for _, d_ in rb], np.int64)))
            wins.append((wchunks, ovA, ovB))
            max_ca = max(max_ca, -(-len(ovA) // P))
            max_cb = max(max_cb, -(-len(ovB) // P))
        per_core_raw.append(wins)

    CA, CB = max_ca, max_cb
    # per-window static dims (shared across cores)
    CA_W = [0] * NW
    CB_W = [1] * NW
    for c in range(NCORES):
        for w, (_, ovA, ovB) in enumerate(per_core_raw[c]):
            CA_W[w] = max(CA_W[w], -(-len(ovA) // P))
            CB_W[w] = max(CB_W[w], -(-len(ovB) // P))
    NCHA_W = [CPW_W[w] * B_A for w in range(NW)]
    NCHB_W = [CPW_W[w] * B_B for w in range(NW)]
    GBA_W = [NCHA_W[w] + CA_W[w] for w in range(NW)]
    GBB_W = [NCHB_W[w] + CB_W[w] for w in range(NW)]
    GB_W = [GBA_W[w] + GBB_W[w] for w in range(NW)]
    OFF_IA = np.concatenate([[0], np.cumsum([g * P // 16 for g in GBA_W])])
    OFF_IB = np.concatenate([[0], np.cumsum([g * P // 16 for g in GBB_W])])
    OFF_DL = np.concatenate([[0], np.cumsum(GB_W)])

    per_core = []
    for c in range(NCORES):
        IA = np.zeros((128, int(OFF_IA[-1])), np.int16)
        IB = np.zeros((128, int(OFF_IB[-1])), np.int16)
        DL = np.full((128, int(OFF_DL[-1])), DUMMY, np.float16)
        for w, (wchunks, ovA, ovB) in enumerate(per_core_raw[c]):
            NCH_A, NCH_B = NCHA_W[w], NCHB_W[w]
            GBA, GB = GBA_W[w], GB_W[w]
            NIA, NIB = GBA_W[w] * P, GBB_W[w] * P
            ia = np.zeros(NIA, np.int16)
            ib = np.zeros(NIB, np.int16)
            dl = np.full((GB, P), DUMMY, np.float32)   # [col, part]
            # chunk columns in (o g) order: col = o*CPW_w + chunk, so the
            # device-side per-chunk broadcasts stay packed on the last axis
            cw = CPW_W[w]
            for m, (ra, dla, rb, dlb) in enumerate(wchunks):
                k = len(ra)
                if k:
                    cols = (np.arange(k) // P) * cw + m
                    parts = np.arange(k) % P
                    ia[cols * P + parts] = ra
                    dl[cols, parts] = dla
                k = len(rb)
                if k:
                    cols = (np.arange(k) // P) * cw + m
                    parts = np.arange(k) % P
                    ib[cols * P + parts] = rb
                    dl[GBA + cols, parts] = dlb
            for k, (rr, d512) in enumerate(ovA):
                col, part = NCH_A + k // P, k % P
                ia[col * P + part] = rr
                dl[col, part] = d512
            for k, (rr, d512) in enumerate(ovB):
                col, part = NCH_B + k // P, k % P
                ib[col * P + part] = rr
                dl[GBA + col, part] = d512
            IA[:, int(OFF_IA[w]):int(OFF_IA[w + 1])] = _wrap16(ia, NIA)
            IB[:, int(OFF_IB[w]):int(OFF_IB[w + 1])] = _wrap16(ib, NIB)
            DL[:, int(OFF_DL[w]):int(OFF_DL[w + 1])] = dl.T.astype(np.float16)
        per_core.append(dict(IA=IA, IB=IB, DL=DL))

    consts = dict(NW=NW, CA=CA, CB=CB, SHARD=SHARD, KT=KT, SHARDP=SHARDP,
                  NT=NT, cut_nodes=cut_nodes, CPW_W=CPW_W, CA_W=CA_W,
                  CB_W=CB_W)
    return per_core, consts


def _build_program(N, cc):
    NW, CA, CB = cc["NW"], cc["CA"], cc["CB"]
    SHARD, KT, SHARDP, NT = cc["SHARD"], cc["KT"], cc["SHARDP"], cc["NT"]
    CPW_W, CA_W, CB_W = cc["CPW_W"], cc["CA_W"], cc["CB_W"]
    NCHA_W = [CPW_W[w] * B_A for w in range(NW)]
    NCHB_W = [CPW_W[w] * B_B for w in range(NW)]
    GBA_W = [NCHA_W[w] + CA_W[w] for w in range(NW)]
    GBB_W = [NCHB_W[w] + CB_W[w] for w in range(NW)]
    GB_W = [GBA_W[w] + GBB_W[w] for w in range(NW)]
    OFF_IA = np.concatenate([[0], np.cumsum([g * P // 16 for g in GBA_W])])
    OFF_IB = np.concatenate([[0], np.cumsum([g * P // 16 for g in GBB_W])])
    OFF_DL = np.concatenate([[0], np.cumsum(GB_W)])
    NCH_A, NCH_B = CPW * B_A, CPW * B_B
    GBA_MX, GBB_MX = max(GBA_W), max(GBB_W)
    GB_MX = max(GB_W)
    KTALL = NT // P
    RNG = NW * WCOLS

    nc = bass.Bass("TRN2", target_bir_lowering=False, debug=False,
                   num_devices=NCORES)

    gb_e = nc.dram_tensor("gate_b", [1], F32, kind="ExternalInput")
    gwt_e = nc.dram_tensor("GWT", [2 * D, 1], BF16, kind="ExternalInput")
    hsht_e = nc.dram_tensor("HSHT", [D, SHARDP], BF16, kind="ExternalInput")
    hrt_e = nc.dram_tensor("HRT", [D, RNG], BF16, kind="ExternalInput")
    htab_e = nc.dram_tensor("HTAB", [NT, ROWE], BF16, kind="ExternalInput")
    ia_e = nc.dram_tensor("IA16", [128, int(OFF_IA[-1])], I16,
                          kind="ExternalInput")
    ib_e = nc.dram_tensor("IB16", [128, int(OFF_IB[-1])], I16,
                          kind="ExternalInput")
    dl_e = nc.dram_tensor("DLW", [128, int(OFF_DL[-1])], F16,
                          kind="ExternalInput")
    dw_e = nc.dram_tensor("DWIN", [RNG], F32, kind="ExternalInput")
    zT_e = nc.dram_tensor("zT", [P, RNG], BF16, kind="ExternalOutput")

    t6_mine = nc.dram_tensor("t6_mine", [SHARDP], BF16)  # projs, my shard
    t6_all = nc.dram_tensor("t6_all", [NCORES * SHARDP], BF16,
                            addr_space="Shared")

    with tile.TileContext(nc) as tc:
        with (
            tc.tile_pool(name="const", bufs=1) as constp,
            tc.tile_pool(name="prol", bufs=1) as prolp,
            tc.tile_pool(name="idx", bufs=8) as idxp,
            tc.tile_pool(name="gh", bufs=2) as ghp,
            tc.tile_pool(name="oh", bufs=2) as ohp,
            tc.tile_pool(name="sc", bufs=2) as scp,
            tc.tile_pool(name="zt", bufs=3) as ztp,
            tc.tile_pool(name="ps", bufs=2, space="PSUM") as psp,
            tc.tile_pool(name="pp", bufs=2, space="PSUM") as ppp,
        ):
            # ---- prologue A load first: it gates projs -> collective ----
            hsht = prolp.tile([P, KT, P], BF16)
            nc.sync.dma_start(
                out=hsht[:],
                in_=hsht_e[:, :].rearrange("d (t p) -> d t p", p=P))

            # ---- constants ----
            zeros_i = constp.tile([P, 1], mybir.dt.int32)
            nc.gpsimd.memset(zeros_i[:], 0)
            bbc = constp.tile([P, 1], F32)        # gate_b bcast
            nc.gpsimd.indirect_dma_start(
                out=bbc[:], out_offset=None, in_=gb_e[:, None],
                in_offset=bass.IndirectOffsetOnAxis(ap=zeros_i[:, :1], axis=0))
            ones_row = constp.tile([1, P], F32)
            nc.gpsimd.memset(ones_row[:], 1.0)
            CMX = max(CA, CB)
            # full iotas varying along the MIDDLE axis so every one-hot
            # operand keeps a packed last dim (DVE 2x mode)
            iotaf = constp.tile([P, CH, NCH_A], F16)
            nc.gpsimd.iota(iotaf[:], pattern=[[1, CH], [0, NCH_A]], base=0,
                           channel_multiplier=0,
                           allow_small_or_imprecise_dtypes=True)
            iota512f = constp.tile([P, WCOLS, CMX], F16)
            nc.gpsimd.iota(iota512f[:], pattern=[[1, WCOLS], [0, CMX]],
                           base=0, channel_multiplier=0,
                           allow_small_or_imprecise_dtypes=True)
            gwt = constp.tile([P, 2, 1], BF16)    # [:,0]=w_dst col [:,1]=w_src
            nc.sync.dma_start(
                out=gwt[:],
                in_=gwt_e[:, :].rearrange("(t p) o -> p t o", p=P))
            zrow = constp.tile([1, WCOLS], F32)
            nc.gpsimd.memset(zrow[:], 0.0)
            nreg = {}                             # gather-size registers
            for g in GBA_W + GBB_W:
                for c0 in range(0, g, 8):
                    n = min(8, g - c0) * P
                    nreg[n] = nreg.get(n) or nc.gpsimd.to_reg(n)

            # ---- prologue A: projs over my shard (PE) -> AllGather ----
            # reuse the zps PSUM tag (prologue-only) to stay within 8 banks
            pjp = psp.tile([P, WCOLS], F32, tag="zps")
            for k in range(KT):
                nc.tensor.matmul(out=pjp[:, k:k + 1], lhsT=hsht[:, k, :],
                                 rhs=gwt[:, 1, :], start=True, stop=True)
            projs_sh = prolp.tile([P, KT], BF16)
            nc.scalar.activation(projs_sh[:], pjp[:, 0:KT], AF.Identity)
            nc.sync.dma_start(
                out=t6_mine[:].rearrange("(p t) -> p t", p=P),
                in_=projs_sh[:])
            nc.gpsimd.collective_compute(
                "AllGather", OP.bypass,
                replica_groups=[list(range(NCORES))],
                ins=[t6_mine[:]], outs=[t6_all[:]])

            # ---- prologue B: write projs column into the host table.
            # Split at SPLIT so grid-A gathers only wait for the first part.
            t6b = prolp.tile([P, KTALL], BF16)
            with tc.high_priority():
                nc.sync.dma_start(
                    out=t6b[:].rearrange("p (s t) -> p s t", t=KT),
                    in_=t6_all[:].rearrange("(s p t) -> p s t", p=P, t=KT))
                KS = SPLIT // P   # row-groups below the grid boundary
                nc.sync.dma_start(
                    out=htab_e[0:SPLIT, D:D + 1]
                    .rearrange("(t p) o -> p t o", p=P),
                    in_=t6b[:, 0:KS].rearrange("p (t o) -> p t o", o=1))
                nc.sync.dma_start(
                    out=htab_e[SPLIT:NT, D:D + 1]
                    .rearrange("(t p) o -> p t o", p=P),
                    in_=t6b[:, KS:KTALL].rearrange("p (t o) -> p t o", o=1))

            # ---- prologue C: resident h^T of my dst range (for projd) ----
            hrt = prolp.tile([P, RNG], BF16)
            nc.sync.dma_start(out=hrt[:], in_=hrt_e[:, :])

            # ---- main loop ----
            for w in range(NW):
                cw = CPW_W[w]
                if cw == 0:
                    continue
                nw_a, nw_b = NCHA_W[w], NCHB_W[w]
                ca, cb = CA_W[w], CB_W[w]
                gba, gbb, gb = GBA_W[w], GBB_W[w], GB_W[w]
                nia, nib = gba * P, gbb * P
                ia = idxp.tile([P, GBA_MX * P // 16], I16, tag="ia")
                nc.sync.dma_start(
                    out=ia[:, 0:nia // 16],
                    in_=ia_e[:, int(OFF_IA[w]):int(OFF_IA[w + 1])])
                ib = idxp.tile([P, GBB_MX * P // 16], I16, tag="ib")
                nc.sync.dma_start(
                    out=ib[:, 0:nib // 16],
                    in_=ib_e[:, int(OFF_IB[w]):int(OFF_IB[w + 1])])
                dlw = idxp.tile([P, GB_MX], F16, tag="dl")
                nc.sync.dma_start(
                    out=dlw[:, 0:gb],
                    in_=dl_e[:, int(OFF_DL[w]):int(OFF_DL[w + 1])])
                dwrow = idxp.tile([1, WCOLS], F32, tag="dw")
                nc.sync.dma_start(out=dwrow[:],
                                  in_=dw_e[None, w * WCOLS:(w + 1) * WCOLS])

                # 512B-row gathers, chunked to <=1024 idxs per call (the
                # device rejects larger SWDGE gather calls)
                GCH = 8
                gh = ghp.tile([P, GB_MX, ROWE], BF16)
                for c0 in range(0, gba, GCH):
                    c1 = min(c0 + GCH, gba)
                    n = (c1 - c0) * P
                    nc.gpsimd.dma_gather(
                        gh[:, c0:c1, :], htab_e[0:SPLIT, :],
                        ia[:, c0 * 8:c1 * 8], n, nreg[n],
                        elem_size=ROWE, elem_step=ROWE)
                for c0 in range(0, gbb, GCH):
                    c1 = min(c0 + GCH, gbb)
                    n = (c1 - c0) * P
                    nc.gpsimd.dma_gather(
                        gh[:, gba + c0:gba + c1, :],
                        htab_e[NT - SPLIT:NT, :],
                        ib[:, c0 * 8:c1 * 8], n, nreg[n],
                        elem_size=ROWE, elem_step=ROWE)

                # projd row for this window: [1,512] = w_dst^T @ h^T slice
                pdr = ppp.tile([1, WCOLS], F32, tag="pdr")
                nc.tensor.matmul(out=pdr[:], lhsT=gwt[:, 0, :],
                                 rhs=hrt[:, w * WCOLS:(w + 1) * WCOLS],
                                 start=True, stop=True)
                pdrb = scp.tile([1, WCOLS], F32, tag="pdrb")
                nc.scalar.activation(pdrb[:], pdr[:], AF.Identity,
                                     bias=bbc[0:1, 0:1])
                pdps = ppp.tile([P, WCOLS], F32, tag="pdps")
                nc.tensor.matmul(out=pdps[:], lhsT=ones_row[:], rhs=pdrb[:],
                                 start=True, stop=True)
                # chunk-transposed [P, c, g] and C-replicated [P, pos, k]
                # copies of the projd row (packed last dims for DVE 2x)
                pdsbT = scp.tile([P, CH, CPW], F16, tag="pdsbT")
                nc.scalar.activation(
                    pdsbT[:], pdps[:].rearrange("p (g c) -> p c g", c=CH),
                    AF.Identity)
                pdc = scp.tile([P, WCOLS, CMX], F16, tag="pdc")
                nc.scalar.activation(
                    pdc[:], pdps[:].rearrange("p (s k) -> p s k", k=1)
                    .to_broadcast([P, WCOLS, CMX]),
                    AF.Identity)

                # one-hots, transposed [P, c, col] (col = o*cw + chunk)
                ohA = ohp.tile([P, CH, NCH_A], F16, tag="ohA")
                nc.vector.tensor_tensor(
                    out=ohA[:, :, 0:nw_a],
                    in0=dlw[:, 0:nw_a].rearrange("p (c l) -> p c l", c=1)
                    .to_broadcast([P, CH, nw_a]),
                    in1=iotaf[:, :, 0:nw_a], op=OP.is_equal)
                ohB = ohp.tile([P, CH, NCH_B], F16, tag="ohB")
                nc.vector.tensor_tensor(
                    out=ohB[:, :, 0:nw_b],
                    in0=dlw[:, gba:gba + nw_b]
                    .rearrange("p (c l) -> p c l", c=1)
                    .to_broadcast([P, CH, nw_b]),
                    in1=iotaf[:, :, 0:nw_b], op=OP.is_equal)
                ohCA = None
                if ca:
                    ohCA = ohp.tile([P, WCOLS, max(CA, 1)], F16, tag="ohCA")
                    nc.vector.tensor_tensor(
                        out=ohCA[:, :, 0:ca],
                        in0=dlw[:, nw_a:gba]
                        .rearrange("p (s k) -> p s k", s=1)
                        .to_broadcast([P, WCOLS, ca]),
                        in1=iota512f[:, :, 0:ca], op=OP.is_equal)
                ohCB = ohp.tile([P, WCOLS, CB], F16, tag="ohCB")
                nc.vector.tensor_tensor(
                    out=ohCB[:, :, 0:cb],
                    in0=dlw[:, gba + nw_b:gb]
                    .rearrange("p (s k) -> p s k", s=1)
                    .to_broadcast([P, WCOLS, cb]),
                    in1=iota512f[:, :, 0:cb], op=OP.is_equal)

                # per-slot projd via one-hot selection
                tselA = ohp.tile([P, CH, NCH_A], F16, tag="tselA")
                nc.vector.tensor_tensor(
                    out=tselA[:, :, 0:nw_a]
                    .rearrange("p c (o g) -> p c o g", o=B_A),
                    in0=ohA[:, :, 0:nw_a]
                    .rearrange("p c (o g) -> p c o g", o=B_A),
                    in1=pdsbT[:, :, 0:cw]
                    .rearrange("p c (o g) -> p c o g", o=1)
                    .to_broadcast([P, CH, B_A, cw]),
                    op=OP.mult)
                qd = scp.tile([P, GB_MX], F32, tag="qd")
                nc.vector.tensor_reduce(
                    out=qd[:, 0:nw_a],
                    in_=tselA[:, :, 0:nw_a].rearrange("p c l -> p l c"),
                    axis=mybir.AxisListType.X, op=OP.add)
                tselB = ohp.tile([P, CH, NCH_B], F16, tag="tselB")
                nc.vector.tensor_tensor(
                    out=tselB[:, :, 0:nw_b]
                    .rearrange("p c (o g) -> p c o g", o=B_B),
                    in0=ohB[:, :, 0:nw_b]
                    .rearrange("p c (o g) -> p c o g", o=B_B),
                    in1=pdsbT[:, :, 0:cw]
                    .rearrange("p c (o g) -> p c o g", o=1)
                    .to_broadcast([P, CH, B_B, cw]),
                    op=OP.mult)
                nc.vector.tensor_reduce(
                    out=qd[:, gba:gba + nw_b],
                    in_=tselB[:, :, 0:nw_b].rearrange("p c l -> p l c"),
                    axis=mybir.AxisListType.X, op=OP.add)
                if ca:
                    tselC = ohp.tile([P, WCOLS, max(CA, 1)], F16,
                                     tag="tselCA")
                    nc.vector.tensor_tensor(
                        out=tselC[:, :, 0:ca], in0=ohCA[:, :, 0:ca],
                        in1=pdc[:, :, 0:ca], op=OP.mult)
                    nc.vector.tensor_reduce(
                        out=qd[:, nw_a:gba],
                        in_=tselC[:, :, 0:ca].rearrange("p s k -> p k s"),
                        axis=mybir.AxisListType.X, op=OP.add)
                tselD = ohp.tile([P, WCOLS, CB], F16, tag="tselCB")
                nc.vector.tensor_tensor(
                    out=tselD[:, :, 0:cb], in0=ohCB[:, :, 0:cb],
                    in1=pdc[:, :, 0:cb], op=OP.mult)
                nc.vector.tensor_reduce(
                    out=qd[:, gba + nw_b:gb],
                    in_=tselD[:, :, 0:cb].rearrange("p s k -> p k s"),
                    axis=mybir.AxisListType.X, op=OP.add)

                # gate: tanh(projd_sel + projs_src) * d_src
                pjs = scp.tile([P, GB_MX], F32, tag="pjs")
                nc.vector.tensor_copy(out=pjs[:, 0:gb],
                                      in_=gh[:, 0:gb, D])
                q = scp.tile([P, GB_MX], F32, tag="q")
                nc.vector.tensor_tensor(out=q[:, 0:gb], in0=qd[:, 0:gb],
                                        in1=pjs[:, 0:gb], op=OP.add)
                g32 = scp.tile([P, GB_MX], F32, tag="g32")
                nc.scalar.activation(g32[:, 0:gb], q[:, 0:gb], AF.Tanh)
                dsc = scp.tile([P, GB_MX], F32, tag="dsc")
                nc.vector.tensor_copy(out=dsc[:, 0:gb],
                                      in_=gh[:, 0:gb, D + 1])
                gb16 = scp.tile([P, GB_MX], F16, tag="gb16")
                nc.vector.tensor_tensor(out=gb16[:, 0:gb],
                                        in0=g32[:, 0:gb], in1=dsc[:, 0:gb],
                                        op=OP.mult)

                # weighted one-hots (bf16, transposed)
                obA = ohp.tile([P, CH, NCH_A], BF16, tag="obA")
                nc.vector.tensor_tensor(
                    out=obA[:, :, 0:nw_a], in0=ohA[:, :, 0:nw_a],
                    in1=gb16[:, 0:nw_a].rearrange("p (c l) -> p c l", c=1)
                    .to_broadcast([P, CH, nw_a]),
                    op=OP.mult)
                obB = ohp.tile([P, CH, NCH_B], BF16, tag="obB")
                nc.vector.tensor_tensor(
                    out=obB[:, :, 0:nw_b], in0=ohB[:, :, 0:nw_b],
                    in1=gb16[:, gba:gba + nw_b]
                    .rearrange("p (c l) -> p c l", c=1)
                    .to_broadcast([P, CH, nw_b]),
                    op=OP.mult)
                obCA = None
                if ca:
                    obCA = ohp.tile([P, WCOLS, max(CA, 1)], BF16, tag="obCA")
                    nc.vector.tensor_tensor(
                        out=obCA[:, :, 0:ca], in0=ohCA[:, :, 0:ca],
                        in1=gb16[:, nw_a:gba]
                        .rearrange("p (s k) -> p s k", s=1)
                        .to_broadcast([P, WCOLS, ca]),
                        op=OP.mult)
                obCB = ohp.tile([P, WCOLS, CB], BF16, tag="obCB")
                nc.vector.tensor_tensor(
                    out=obCB[:, :, 0:cb], in0=ohCB[:, :, 0:cb],
                    in1=gb16[:, gba + nw_b:gb]
                    .rearrange("p (s k) -> p s k", s=1)
                    .to_broadcast([P, WCOLS, cb]),
                    op=OP.mult)

                # scatter matmuls (zero-init first, then accumulate)
                zps = psp.tile([P, WCOLS], F32, tag="zps")
                nc.tensor.matmul(out=zps[:], lhsT=ones_row[:], rhs=zrow[:],
                                 start=True, stop=False,
                                 skip_group_check=True)
                for k in range(ca):
                    nc.tensor.matmul(out=zps[:],
                                     lhsT=gh[:, nw_a + k, 0:D],
                                     rhs=obCA[:, :, k],
                                     start=False, stop=False,
                                     skip_group_check=True)
                for k in range(cb):
                    nc.tensor.matmul(out=zps[:],
                                     lhsT=gh[:, gba + nw_b + k, 0:D],
                                     rhs=obCB[:, :, k],
                                     start=False, stop=False,
                                     skip_group_check=True)
                for m in range(cw):
                    for j in range(B_A):
                        col = j * cw + m
                        nc.tensor.matmul(
                            out=zps[:, m * CH:(m + 1) * CH],
                            lhsT=gh[:, col, 0:D], rhs=obA[:, :, col],
                            start=False, stop=False, skip_group_check=True)
                    for j in range(B_B):
                        col = j * cw + m
                        nc.tensor.matmul(
                            out=zps[:, m * CH:(m + 1) * CH],
                            lhsT=gh[:, gba + col, 0:D], rhs=obB[:, :, col],
                            start=False,
                            stop=(m == cw - 1 and j == B_B - 1),
                            skip_group_check=True)

                # d_dst column scale + store
                dwps = ppp.tile([P, WCOLS], F32, tag="dwps")
                nc.tensor.matmul(out=dwps[:], lhsT=ones_row[:], rhs=dwrow[:],
                                 start=True, stop=True)
                dws = ztp.tile([P, WCOLS], F32, tag="dws")
                nc.scalar.activation(dws[:], dwps[:], AF.Identity)
                zsb = ztp.tile([P, WCOLS], BF16, tag="zsb")
                nc.vector.tensor_tensor(out=zsb[:], in0=zps[:], in1=dws[:],
                                        op=OP.mult)
                nc.sync.dma_start(out=zT_e[:, w * WCOLS:(w + 1) * WCOLS],
                                  in_=zsb[:])
    _fix_library_reloads(nc)
    _split_multiwait(nc)
    return nc


def _host_inputs(h, d, gate_w, gate_b, per_core, cc, N):
    """Pack host-side inputs (pure relayout/cast of problem inputs)."""
    NW, SHARD, SHARDP, NT = cc["NW"], cc["SHARD"], cc["SHARDP"], cc["NT"]
    cut = cc["cut_nodes"]
    RNG = NW * WCOLS

    # remap-order index: table row r -> node
    s_ids = np.arange(NCORES).repeat(SHARDP)
    offs = np.tile(np.arange(SHARDP), NCORES)
    nodes = np.minimum(s_ids * SHARD + np.minimum(offs, SHARD - 1), N - 1)

    HTAB = np.zeros((NT, ROWE), ml_dtypes.bfloat16)
    HTAB[:, 0:D] = h[nodes].astype(ml_dtypes.bfloat16)
    HTAB[:, D + 1] = d[nodes].astype(ml_dtypes.bfloat16)
    GWT = gate_w[0].astype(ml_dtypes.bfloat16).reshape(2 * D, 1)

    in_maps = []
    for c in range(NCORES):
        n0 = int(cut[c])
        rng_nodes = np.minimum(n0 + np.arange(RNG), N - 1)
        HRT = np.ascontiguousarray(
            h[rng_nodes].astype(ml_dtypes.bfloat16).T)
        sh_nodes = np.minimum(c * SHARD + np.minimum(
            np.arange(SHARDP), SHARD - 1), N - 1)
        HSHT = np.ascontiguousarray(
            h[sh_nodes].astype(ml_dtypes.bfloat16).T)
        dwin = np.zeros(RNG, np.float32)
        span = min(RNG, N - n0)
        dwin[:span] = d[n0:n0 + span]
        pc = per_core[c]
        in_maps.append({
            "gate_b": gate_b.astype(np.float32),
            "GWT": GWT, "HSHT": HSHT, "HRT": HRT, "HTAB": HTAB,
            "IA16": pc["IA"], "IB16": pc["IB"], "DLW": pc["DL"],
            "DWIN": dwin,
        })
    return in_maps


def _run(h, d, src, dst, gate_w, gate_b, trace=False, tmpdir=None):
    N = h.shape[0]
    h = np.ascontiguousarray(np.asarray(h, dtype=np.float32))
    d = np.ascontiguousarray(np.asarray(d, dtype=np.float32))
    gate_w = np.ascontiguousarray(np.asarray(gate_w, dtype=np.float32))
    gate_b = np.ascontiguousarray(np.asarray(gate_b, dtype=np.float32))

    per_core, cc = _preprocess(np.asarray(src), np.asarray(dst), N)
    nc = _build_program(N, cc)
    in_maps = _host_inputs(h, d, gate_w, gate_b, per_core, cc, N)

    res = run_bass_kernel_spmd(nc, in_maps, core_ids=list(range(NCORES)),
                               trace=trace, tmpdir=tmpdir)
    cut = cc["cut_nodes"]
    z = np.empty((N, D), dtype=np.float32)
    for c in range(NCORES):
        n0, n1 = int(cut[c]), int(cut[c + 1])
        z[n0:n1, :] = res.results[c]["zT"][:, :n1 - n0].T.astype(np.float32)
    return z, res


def kernel(h, d, src, dst, gate_w, gate_b):
    z, _ = _run(h, d, src, dst, gate_w, gate_b)
    return z
